# revision 35
# baseline (speedup 1.0000x reference)
"""CantorAttention Trainium2 kernel (8 NeuronCores, SPMD), v2.

Strategy
--------
Shard (batch=2) x (head-pairs=4) across the 8 cores: core c handles batch
c//4 and heads {2*(c%4), 2*(c%4)+1}.  QKV projection is column-sharded,
output projection row-sharded per head pair; partial outputs are summed on
host.

The sparse gather `k[:, :, routes, :]` becomes dense band attention after a
host-side sort of positions by Cantor coordinate: each 128-query tile's 64
routed keys then span < 256 consecutive sorted positions, so every tile
needs exactly two 128-key chunks at an ARBITRARY (unaligned) window start.
The unaligned key window is handled in the PV matmul by splitting the
contraction across two partition sub-ranges of the natural-layout V.
A 0/1 mask (exact; routes are distinct per query) multiplies exp(scores).

Device dataflow per core (bf16 compute, f32 PSUM accumulate):
  stage A   qkvT = Wqkv_c.T @ xT  per 512-col quarter  -> qT|kT|vT in SBUF
  stage B   v natural chunks via PE transpose of vT (+ ones cols for Z)
  stage C   per query tile t (window [w, w+256)):
              S^T = kT[:, w+128c].T @ qT[:, tile]   (4 matmuls -> 1 bank)
              P^T = exp(S*scale) (ACT) * mask (DVE)
              attn|Z = P^T.T @ [V|1]   (8 matmuls, split partition ranges)
              attn *= 1/Z  (recip + 2 tensor_scalar muls)
            per group of 4 tiles: aT = attn^T (PE), out^T = Wout-chunks @ aT
  DMA out^T per tile-group; host un-permutes, sums partials, adds biases.

Elementwise work is spread over ACT, DVE, and Pool (gpsimd); DMAs are
issued from multiple sequencers with layouts made fully contiguous by the
host so each transfer is a handful of large descriptors.
"""

import numpy as np
import ml_dtypes

import concourse.bass as bass
import concourse.tile as tile
from concourse import bacc, mybir, masks
from concourse.bass_utils import run_bass_kernel_spmd

BF16 = ml_dtypes.bfloat16
B, S, DIM, H, HD, KNN = 2, 2048, 512, 8, 64, 64
NCORES = 8
T = 128            # queries per tile
NT = S // T        # 16
CCH = DIM // 128   # 4 contraction chunks of the model dim
QTR = 512          # stage-A quarter width
NQR = S // QTR     # 4
SCALE = 1.0 / float(np.sqrt(HD))
VSTR = 132         # v block stride: [v_h0 64 | ones 2 | v_h1 64 | ones 2]
TRUNC = 6                # build prefix level for debugging (6 = full)
PE_WARMUP = 28           # dependency-free PE transposes before stage A
USE_GPSIMD_MASK = True   # Pool engine takes a share of the mask multiplies
USE_ACT_DMA = True       # issue small input DMAs from the ACT sequencer
USE_DMA_VTRANS = False   # XBAR needs contiguous dst; V slots are strided
USE_DMA_ATRANS = True    # blocked XBAR DMA transpose for attn -> aT
USE_BCAST_NORM = True    # stride-0 broadcast AP in the normalize mul
MULTI_START_PSUM = True  # single start flag per PSUM bank for scores


# ----------------------------------------------------------------------------
# Host-side planning: permutation + per-tile unaligned key windows + masks
# ----------------------------------------------------------------------------

def _cantor_perm() -> np.ndarray:
    x = np.arange(S, dtype=np.float64) / max(1, S - 1)
    x = np.clip(x, 1e-06, 1.0 - 1e-06)
    val = np.zeros(S, dtype=np.float64)
    factor = 0.5
    for _ in range(8):
        x *= 3.0
        digit = np.floor(x)
        x -= digit
        val += (digit == 2.0) * factor
        factor *= 0.5
    return np.argsort(val.astype(np.float32), kind="stable")


class Plan:
    pass


def _plan(routes: np.ndarray) -> Plan:
    routes = np.asarray(routes)
    candidates = [
        _cantor_perm(),
        np.arange(S),
        np.argsort(routes.min(axis=1), kind="stable"),
        np.argsort(np.median(routes, axis=1), kind="stable"),
    ]
    best = None
    for perm in candidates:
        inv = np.empty(S, np.int64)
        inv[perm] = np.arange(S)
        r_q = inv[routes][perm]
        nkc = []
        for t in range(NT):
            blk = r_q[t * T:(t + 1) * T]
            w64 = (int(blk.min()) // 64) * 64
            span = int(blk.max()) + 1 - w64
            nkc.append(-(-span // T))
        cost = sum(nkc)
        if best is None or cost < best[0]:
            best = (cost, perm, r_q, nkc)
    _, perm, r_q, nkc = best

    p = Plan()
    p.perm = perm
    p.nkc = []
    p.w = []
    # jobs: one scores+exp+mask unit covering <=2 key chunks of one tile.
    # mask layout (dup for both heads): [h0: c0 c1 | h1: c0 c1]
    # window starts are 64-aligned so every 128-key chunk is a full
    # 128-partition block of either v_even (offset 0) or v_odd (offset 64).
    for t in range(NT):
        blk = r_q[t * T:(t + 1) * T]
        w64 = (int(blk.min()) // 64) * 64
        span = int(blk.max()) + 1 - w64
        nk = -(-span // T)
        p.nkc.append(nk)
        p.w.append(min(w64, S - T * nk))
    # group identical chunk starts shared by consecutive tiles: one scores
    # matmul per (chunk, head) covering all sharing tiles' query columns
    chunk_tiles = {}
    for t in range(NT):
        for ci in range(p.nkc[t]):
            chunk_tiles.setdefault(p.w[t] + ci * T, []).append(t)
    jobs = []            # (chunk_start, t0, ntile, mask_off)
    p.job_of = {}        # (t, ci) -> job index
    mask_cols = 0
    for cs in sorted(chunk_tiles):
        ts = sorted(chunk_tiles[cs])
        runs = []
        for t in ts:
            if runs and t == runs[-1][-1] + 1:
                runs[-1].append(t)
            else:
                runs.append([t])
        for run in runs:
            i = 0
            while i < len(run):
                ntile = min(3, len(run) - i)
                jidx = len(jobs)
                for t in run[i:i + ntile]:
                    ci = (cs - p.w[t]) // T
                    p.job_of[(t, ci)] = jidx
                jobs.append((cs, run[i], ntile, mask_cols))
                mask_cols += 2 * ntile * T
                i += ntile
    p.jobs = jobs
    p.mask_cols = mask_cols
    # which even (key//128) / odd ((key-64)//128) v blocks are referenced
    p.even_used, p.odd_used = set(), set()
    for t in range(NT):
        for ci in range(p.nkc[t]):
            wk = p.w[t] + ci * T
            if wk % 128 == 0:
                p.even_used.add(wk // 128)
            else:
                p.odd_used.add((wk - 64) // 128)

    maskA = np.zeros((T, mask_cols), np.float32)
    for (cs, t0, ntile, moff) in jobs:
        for j, t in enumerate(range(t0, t0 + ntile)):
            blk = r_q[t * T:(t + 1) * T]      # (T queries, K keys)
            qidx = np.broadcast_to(np.arange(T)[:, None], blk.shape)
            sel = (blk >= cs) & (blk < cs + T)
            ki = blk[sel] - cs
            qi = qidx[sel]
            maskA[ki, moff + j * T + qi] = 1.0
            maskA[ki, moff + (ntile + j) * T + qi] = 1.0
    p.maskA = np.ascontiguousarray(maskA.astype(BF16))
    return p


# ----------------------------------------------------------------------------
# Device program
# ----------------------------------------------------------------------------

def _build(p: Plan, with_qk_bias: bool):
    f32 = mybir.dt.float32
    bf16 = mybir.dt.bfloat16
    nc = bacc.Bacc("TRN2", target_bir_lowering=False, debug=False,
                   num_devices=NCORES)

    # HBM layouts are exactly the SBUF layouts -> contiguous DMAs.
    xT_d = nc.dram_tensor("xT", [CCH, NQR, 128, QTR], bf16,
                          kind="ExternalInput").ap()
    wqkv_d = nc.dram_tensor("wqkv", [128, CCH * 384], bf16,
                            kind="ExternalInput").ap()
    wout_d = nc.dram_tensor("wout", [128, DIM], bf16,
                            kind="ExternalInput").ap()
    maskA_d = nc.dram_tensor("maskA", [128, p.mask_cols], bf16,
                             kind="ExternalInput").ap()
    if with_qk_bias:
        bqk_d = nc.dram_tensor("bqk", [256, 1], f32, kind="ExternalInput").ap()
    outT_d = nc.dram_tensor("outT", [NT // 4, 128, CCH * QTR], bf16,
                            kind="ExternalOutput").ap()

    with tile.TileContext(nc) as tc:
        NSLOT = 2 * NT - 1  # 16 even + 15 odd key blocks per head
        with (
            tc.tile_pool(name="persist", bufs=1) as persist,
            tc.tile_pool(name="pp1", bufs=2, space="PSUM") as pp1,   # A + proj
            tc.tile_pool(name="pp2", bufs=2, space="PSUM") as pp2,   # scores
            tc.tile_pool(name="pp3", bufs=2, space="PSUM") as pp3,   # pso/pst
            tc.tile_pool(name="ptp", bufs=6) as ptp,
            tc.tile_pool(name="attn4p", bufs=2) as attn4p,
            tc.tile_pool(name="rzp", bufs=4) as rzp,
            tc.tile_pool(name="aTp", bufs=2) as aTp,
        ):
            xT = persist.tile([128, CCH * S], bf16, tag="xT")
            qkT = persist.tile([128, 2 * S], bf16, tag="qkT")
            vT = persist.tile([128, S], bf16, tag="vT")
            # v natural key blocks, head-major: head h slot g at
            # (h*NSLOT+g)*66, layout [v 64 | ones 1 | pad 1].
            # slots 0..NT-1: keys [128a, 128a+128); NT..: [64+128j, 192+128j)
            v_sb = persist.tile([128, 2 * NSLOT * 66], bf16, tag="v")
            wqkv = persist.tile([128, CCH * 384], bf16, tag="wqkv")
            wout = persist.tile([128, DIM], bf16, tag="wout")
            maskA = persist.tile([128, p.mask_cols], bf16, tag="maskA")
            outT = persist.tile([128, CCH * S], bf16, tag="outT")
            ident = persist.tile([128, 128], bf16, tag="ident")

            masks.make_identity(nc, ident[:])
            nc.vector.memset(
                v_sb[:].rearrange("p (g f) -> p g f",
                                  g=2 * NSLOT)[:, :, 64:65], 1.0)

            # ---- input DMAs (ACT issues the early small ones; SP does xT
            # halves so stage A can start ~2.5us in) ----
            dma_eng = nc.scalar if USE_ACT_DMA else nc.sync
            # wqkv first (every matmul needs it), then xT half-by-half with
            # the two HWDGE sequencers issuing in parallel
            dma_eng.dma_start(wqkv[:], wqkv_d)
            xT3 = xT[:].rearrange("p (c n f) -> p c n f", c=CCH, n=NQR)
            xt_engs = [nc.sync, dma_eng, nc.gpsimd, nc.sync]
            for half in range(2):
                for c in range(CCH):
                    xt_engs[c].dma_start(
                        xT3[:, c, 2 * half:2 * half + 2],
                        xT_d[c, 2 * half:2 * half + 2].rearrange(
                            "n p f -> p n f"))
            mq = -(-p.mask_cols // (2 * 512)) * 512
            for mo in range(0, p.mask_cols, mq):
                hi = min(mo + mq, p.mask_cols)
                dma_eng.dma_start(maskA[:, mo:hi], maskA_d[:, mo:hi])
            nc.sync.dma_start(wout[:], wout_d)
            if with_qk_bias:
                bqk = persist.tile([128, 2], f32, tag="bqk")
                nc.sync.dma_start(
                    bqk[:].rearrange("p (c f) -> p c f", c=2),
                    bqk_d.rearrange("(c p) f -> p c f", p=128))

            # round-robin between the two PSUM-capable engines
            rr_state = [0]
            ENGS = [nc.vector, nc.scalar]

            def rr():
                e = ENGS[rr_state[0] % 2]
                rr_state[0] += 1
                return e

            def copy_to(eng, dst, src):
                if eng is nc.scalar:
                    nc.scalar.copy(dst, src)
                else:
                    eng.tensor_copy(dst, src)

            # ---- emission helpers ----
            def emit_Av(nq):
                qs = nq * QTR
                ps = pp1.tile([128, QTR], f32, tag="ps1")
                for c in range(CCH):
                    nc.tensor.matmul(
                        ps[:],
                        lhsT=wqkv[:, c * 384 + 256:c * 384 + 384],
                        rhs=xT[:, c * S + qs:c * S + qs + QTR],
                        start=(c == 0), stop=(c == CCH - 1))
                nc.vector.tensor_copy(vT[:, qs:qs + QTR], ps[:])


            def emit_Aqk(nq):
                qs = nq * QTR
                for f in range(2):
                    ps = pp1.tile([128, QTR], f32, tag="ps1")
                    for c in range(CCH):
                        nc.tensor.matmul(
                            ps[:],
                            lhsT=wqkv[:, c * 384 + f * 128:
                                      c * 384 + (f + 1) * 128],
                            rhs=xT[:, c * S + qs:c * S + qs + QTR],
                            start=(c == 0), stop=(c == CCH - 1))
                    dst = qkT[:, f * S + qs:f * S + qs + QTR]
                    if with_qk_bias:
                        nc.vector.tensor_scalar_add(dst, ps[:],
                                                    bqk[:, f:f + 1])
                    else:
                        nc.vector.tensor_copy(dst, ps[:])

            def emit_vt_pe(slot, col0):
                psv = pp3.tile([128, 136], bf16, tag="po")
                nc.tensor.transpose(psv[:, 0:128],
                                    vT[:, col0:col0 + 128], ident[:])
                dst = v_sb[:].rearrange(
                    "p (h rest) -> p h rest",
                    h=2)[:, :, slot * 66:slot * 66 + 64]
                nc.scalar.copy(dst, psv[:, 0:128].rearrange(
                    "p (h f) -> p h f", h=2))

            def emit_vtrans():
                if not USE_DMA_VTRANS:
                    for g in range(NT):
                        if g in p.even_used:
                            emit_vt_pe(g, g * 128)
                    for j in range(NT - 1):
                        if j in p.odd_used:
                            emit_vt_pe(NT + j, 64 + 128 * j)
                    return
                # blocked DMA transposes build all natural-V key blocks:
                # v_sb[p, slot g, hd f] = vT[h*64+f, keybase(g)+p]
                for h in range(2):
                    base = h * NSLOT
                    ev = v_sb[:, base * 66:(base + NT) * 66].rearrange(
                        "p (g f) -> p g f", g=NT)[:, :, 0:64]
                    (nc.scalar if USE_ACT_DMA else nc.sync
                     ).dma_start_transpose(ev, vT[h * 64:(h + 1) * 64, 0:S])
                    od = v_sb[:, (base + NT) * 66:(base + NSLOT) * 66
                              ].rearrange("p (g f) -> p g f",
                                          g=NT - 1)[:, :, 0:64]
                    (nc.scalar if USE_ACT_DMA else nc.sync
                     ).dma_start_transpose(od,
                                           vT[h * 64:(h + 1) * 64, 64:S - 64])

            pt_tiles = {}
            attn4_cur = [None]
            attn4_tiles = {}

            def emit_scores(jidx):
                cs, t0, ntile, moff = p.jobs[jidx]
                qw = ntile * T
                ncols = 2 * qw
                # one PSUM bank per head: all matmuls touching a bank share
                # the same PE tile_position (mixing row bases in one bank
                # wedges the device)
                pss = pp2.tile([128, 1024], f32, tag="pss")
                for h in range(2):
                    nc.tensor.matmul(
                        pss[:, h * 512:h * 512 + qw],
                        lhsT=qkT[h * 64:(h + 1) * 64, S + cs:S + cs + T],
                        rhs=qkT[h * 64:(h + 1) * 64, t0 * T:t0 * T + qw],
                        start=True, stop=True)
                pt = ptp.tile([128, 768], bf16, tag="pt")
                nc.scalar.activation(
                    pt[:, 0:ncols].rearrange("p (h f) -> p h f", h=2),
                    pss[:].rearrange("p (h f) -> p h f", h=2)[:, :, 0:qw],
                    mybir.ActivationFunctionType.Exp, scale=SCALE)
                meng = (nc.gpsimd if (USE_GPSIMD_MASK and jidx % 2 == 1)
                        else nc.vector)
                meng.tensor_mul(pt[:, 0:ncols], pt[:, 0:ncols],
                                maskA[:, moff:moff + ncols])
                pt_tiles[jidx] = pt

            pso_cur = [None]

            def emit_pv(t):
                w = p.w[t]
                nk = p.nkc[t]
                if t % 2 == 0:
                    pso_cur[0] = pp3.tile([128, 272], f32, tag="po",
                                          name="pso2")
                pso = pso_cur[0][:, (t % 2) * 136:(t % 2) * 136 + 136]
                nmm = 0
                tot = 2 * nk
                for h in range(2):
                    for ci in range(nk):
                        jidx = p.job_of[(t, ci)]
                        cs_j, t0_j, ntile_j, moff = p.jobs[jidx]
                        pt = pt_tiles[jidx]
                        lcol = (h * ntile_j + (t - t0_j)) * T
                        wk = w + ci * T
                        if wk % 128 == 0:
                            slot = wk // 128
                        else:
                            slot = NT + (wk - 64) // 128
                        vcol = (h * NSLOT + slot) * 66
                        if MULTI_START_PSUM:
                            st, sp = nmm == 0, nmm == tot - 1
                        else:
                            st, sp = ci == 0, ci == nk - 1
                        nc.tensor.matmul(
                            pso[:, h * 68:h * 68 + 65],
                            lhsT=pt[:, lcol:lcol + T],
                            rhs=v_sb[:, vcol:vcol + 65],
                            start=st, stop=sp)
                        nmm += 1
                if t % 4 == 0:
                    attn4_cur[0] = attn4p.tile([128, 512], bf16, tag="attn4", name="attn4")
                    attn4_tiles[t // 4] = attn4_cur[0]
                attn4 = attn4_cur[0]
                rz = rzp.tile([128, 2], f32, tag="rz")
                nc.vector.reciprocal(
                    rz[:],
                    pso[:].rearrange("p (h f) -> p h f", h=2)[:, :, 64:65])
                if USE_BCAST_NORM:
                    rz_ap = rz[:]
                    rz_b = bass.AP(rz_ap.tensor, rz_ap.offset,
                                   [list(rz_ap.ap[0]), list(rz_ap.ap[1]),
                                    [0, 64]])
                    dst3 = attn4[:, (t % 4) * 128:(t % 4 + 1) * 128
                                 ].rearrange("p (h f) -> p h f", h=2)
                    src3 = pso[:].rearrange(
                        "p (h f) -> p h f", h=2)[:, :, 0:64]
                    nc.vector.tensor_tensor(dst3, src3, rz_b,
                                            mybir.AluOpType.mult)
                else:
                    c0 = (t % 4) * 128
                    nc.vector.tensor_scalar_mul(
                        attn4[:, c0:c0 + 64], pso[:, 0:64], rz[:, 0:1])
                    nc.scalar.mul(
                        attn4[:, c0 + 64:c0 + 128], pso[:, 68:132],
                        rz[:, 1:2])

            def emit_group(tg):
                attn4 = attn4_tiles[tg]
                aT = aTp.tile([128, 512], bf16, tag="aT")
                if USE_DMA_ATRANS and tg < NT // 4 - 1:
                    # blocked DMA transpose: aT[p,g,f] = attn4[f, g*128+p]
                    nc.sync.dma_start_transpose(
                        aT[:].rearrange("p (g f) -> p g f", g=4), attn4[:])
                else:
                    # last group: PE transposes keep the kernel tail short
                    for j in range(4):
                        pst = pp3.tile([128, 136], bf16, tag="po")
                        nc.tensor.transpose(
                            pst[:, 0:128],
                            attn4[:, j * 128:(j + 1) * 128], ident[:])
                        copy_to(rr(), aT[:, j * 128:(j + 1) * 128],
                                pst[:, 0:128])
                last = tg == NT // 4 - 1
                for oc in range(CCH):
                    psp = pp1.tile([128, QTR], f32, tag="ps1")
                    nc.tensor.matmul(
                        psp[:], lhsT=wout[:, oc * 128:(oc + 1) * 128],
                        rhs=aT[:], start=True, stop=True)
                    copy_to(rr(), outT[:, tg * 2048 + oc * QTR:
                                       tg * 2048 + (oc + 1) * QTR], psp[:])
                    if last:
                        nc.sync.dma_start(
                            outT_d[tg].rearrange(
                                "p (o f) -> p o f", o=CCH)[:, oc],
                            outT[:, tg * 2048 + oc * QTR:
                                 tg * 2048 + (oc + 1) * QTR])
                if not last:
                    nc.sync.dma_start(outT_d[tg],
                                      outT[:, tg * 2048:(tg + 1) * 2048])

            # ---- static interleaved schedule with bounded pipeline depth ----
            # Emission order IS per-engine execution order.  Scores run ahead
            # of PV by DEPTH tiles so the ACT exp + DVE mask chain of tile t
            # completes while the PE works on tiles t+1..t+DEPTH.  The pt pool
            # (4 bufs) caps scored-but-unconsumed jobs at 3.
            NJOBS = len(p.jobs)
            sc_next = [0]    # next job index to emit scores for
            pv_next = [0]    # next tile to emit PV for
            grouped = set()
            PT_BUFS, DEPTH = 6, 2

            def job_ready(jidx, kcov):
                cs, t0, ntile, _ = p.jobs[jidx]
                return max(cs + T, (t0 + ntile) * T) <= kcov

            def tile_scored(t):
                return all(p.job_of[(t, ci)] < sc_next[0]
                           for ci in range(p.nkc[t]))

            def jobs_pved():
                # leading jobs whose covering tiles are all PVed (pt free)
                n = 0
                for (cs, t0, ntile, _) in p.jobs:
                    if t0 + ntile - 1 < pv_next[0]:
                        n += 1
                    else:
                        break
                return n

            def pv_ready(vcov):
                t = pv_next[0]
                return (t < NT and tile_scored(t)
                        and p.w[t] + p.nkc[t] * T <= vcov)

            def emit_pv_step():
                t = pv_next[0]
                emit_pv(t)
                pv_next[0] += 1
                if TRUNC < 5:
                    return
                # group with a one-tile lag so the last tile's normalize
                # has drained before its PE transpose
                for tg in range(NT // 4):
                    if tg in grouped:
                        continue
                    if pv_next[0] >= tg * 4 + 5 or pv_next[0] == NT:
                        if tg * 4 + 4 <= pv_next[0]:
                            emit_group(tg)
                            grouped.add(tg)

            def pump(kcov, vcov):
                while True:
                    lead = sc_next[0] - jobs_pved()
                    can_sc = (sc_next[0] < NJOBS
                              and job_ready(sc_next[0], kcov)
                              and lead < PT_BUFS - 1)
                    can_pv = pv_ready(vcov)
                    if can_pv and (lead >= DEPTH or not can_sc):
                        emit_pv_step()
                    elif can_sc:
                        emit_scores(sc_next[0])
                        sc_next[0] += 1
                    elif can_pv:
                        emit_pv_step()
                    else:
                        break

            # PE warm-up: the tensor engine p-state ramps only while
            # continuously busy; burn idle DMA-wait time on dependency-free
            # transposes so real matmuls start at full clock.
            for _ in range(PE_WARMUP):
                psw = pp3.tile([128, 136], bf16, tag="po", name="psw")
                nc.tensor.transpose(psw[:, 0:128], ident[:], ident[:])

            # quarter-paced: each quarter's q/k/v projections, then the v
            # blocks whose source columns are now available, then as much of
            # stage C as is enabled.
            slots_todo = []
            for g in sorted(p.even_used):
                slots_todo.append((g * 128 + 128, g, g * 128))
            for j in sorted(p.odd_used):
                slots_todo.append((192 + 128 * j, NT + j, 64 + 128 * j))
            slots_todo.sort()
            slot_i = [0]

            def emit_vt_ready(cov):
                while (slot_i[0] < len(slots_todo)
                       and slots_todo[slot_i[0]][0] <= cov):
                    _, slot, col0 = slots_todo[slot_i[0]]
                    emit_vt_pe(slot, col0)
                    slot_i[0] += 1

            for nq in range(NQR):
                emit_Av(nq)
                emit_Aqk(nq)
                cov = QTR * (nq + 1)
                if TRUNC >= 2:
                    emit_vt_ready(cov)
                if TRUNC >= 3 and nq > 0:
                    pump(cov, cov if TRUNC >= 4 else 0)
            if TRUNC >= 3:
                pump(S, S if TRUNC >= 4 else 0)
            if TRUNC >= 6:
                assert sc_next[0] == NJOBS and pv_next[0] == NT, (
                    sc_next, pv_next)
                assert len(grouped) == NT // 4
            if TRUNC < 6:
                # flush a dummy output so the harness has bytes to fetch
                if TRUNC < 5:
                    nc.vector.memset(outT[:], 0.0)
                for tg in range(NT // 4):
                    nc.sync.dma_start(outT_d[tg],
                                      outT[:, tg * 2048:(tg + 1) * 2048])

    nc.compile()
    return nc


_CACHE = {}


def _get_program(p: Plan, with_qk_bias: bool):
    key = (tuple(p.w), tuple(p.nkc), p.mask_cols, bool(with_qk_bias))
    if key not in _CACHE:
        _CACHE[key] = _build(p, with_qk_bias)
    return _CACHE[key]


# ----------------------------------------------------------------------------
# Entry point
# ----------------------------------------------------------------------------

def kernel(x, Wqkv, bqkv, Wout, bout, routes):
    x = np.asarray(x, np.float32)
    Wqkv = np.asarray(Wqkv, np.float32)
    bqkv = np.asarray(bqkv, np.float32)
    Wout = np.asarray(Wout, np.float32)
    bout = np.asarray(bout, np.float32)
    routes = np.asarray(routes)

    p = _plan(routes)
    perm = p.perm

    bq = bqkv[0:DIM]
    bk = bqkv[DIM:2 * DIM]
    bv = bqkv[2 * DIM:3 * DIM]
    with_qk_bias = bool(np.any(bq) or np.any(bk))

    nc = _get_program(p, with_qk_bias)

    in_maps = []
    for c in range(NCORES):
        b = c // 4
        h0 = 2 * (c % 4)
        cols = slice(h0 * HD, (h0 + 2) * HD)
        wqkv = np.concatenate(
            [Wqkv[:, cols],
             Wqkv[:, DIM + h0 * HD:DIM + (h0 + 2) * HD],
             Wqkv[:, 2 * DIM + h0 * HD:2 * DIM + (h0 + 2) * HD]], axis=1)
        # SBUF layout [128, c*384 + f*128 + i] = wqkv[c*128 + p, f*128 + i]
        wq = np.ascontiguousarray(
            wqkv.reshape(CCH, 128, 384).transpose(1, 0, 2).reshape(
                128, CCH * 384)).astype(BF16)
        xt = x[b].T[:, perm]  # (512, 2048)
        xt4 = np.ascontiguousarray(
            xt.reshape(CCH, 128, NQR, QTR).transpose(0, 2, 1, 3)).astype(BF16)
        m = {
            "xT": xt4,
            "wqkv": wq,
            "wout": np.ascontiguousarray(
                Wout[h0 * HD:(h0 + 2) * HD, :]).astype(BF16),
            "maskA": p.maskA,
        }
        if with_qk_bias:
            m["bqk"] = np.concatenate(
                [bq[h0 * HD:(h0 + 2) * HD],
                 bk[h0 * HD:(h0 + 2) * HD]]).reshape(256, 1).astype(np.float32)
        in_maps.append(m)

    global _last_in_maps
    _last_in_maps = in_maps
    res = run_bass_kernel_spmd(nc, in_maps, core_ids=list(range(NCORES)))

    out = np.zeros((B, S, DIM), np.float32)
    for c in range(NCORES):
        b = c // 4
        # outT_d[tg, p, oc*512 + s] = out^T[oc*128 + p, tg*512 + s]
        ot = res.results[c]["outT"].astype(np.float32)  # (4, 128, 2048)
        full = ot.reshape(4, 128, CCH, QTR).transpose(2, 1, 0, 3).reshape(
            DIM, S)
        out[b][perm] += full.T
    out += bout[None, None, :]
    if np.any(bv):
        out += (bv @ Wout)[None, None, :]
    return out
